# revision 12
# baseline (speedup 1.0000x reference)
"""Trainium2 Bass kernel for CLSProcess: diagonal linear recurrence
state_t = y_t * state_{t-1} + x_t * z_t over [B=8, T=4096, units=1024].

Sharding: batch across the 8 cores (one batch element per core); the
recurrence is handled per-core with a chunked scan:
  - time is cut into 32 blocks of L=128 steps (partition dim = time)
  - per block, the decay matrix M'[t,s] = x_s * prod_{r=s+1..t} y_r
    (0 for s>t) is built EXACTLY with a DVE tensor_tensor_scan over the
    identity: state_s(t) = y_t*state + I[s==t]  =>  out[s,t] = M[t,s]
    (the lhsT layout the PE matmul wants), then scaled per-partition by
    x_s (x is staged time-major in the partition dim by the host).
    Scans are batched 4 blocks per instruction ([128,512]) with the y
    at block boundaries zeroed so the running state resets at each
    block start. The scan pipeline runs in bf16 (fp32 internal state)
    for 2x DVE throughput.
  - block output = M' @ z  (PE matmul, bf16 operands, fp32 PSUM) +
    carry term
  - carry term: po[t,:] += p_t * prev[127,:] with
    p_t = prod_{r=block_start..t} y_r = y_block_start * mt4raw[0, t]
    (no separate scan needed). Engines can only address partition
    bases {0,32,64,96} and matmul bases {0,32,64}, so row 127 of the
    previous block is reached with a K=64 matmul: sel[s,t] =
    I[s==127] * p_t (rows [64:128] used) against prev[64:128,:].
  - engine budget (cost = free-dim size x cycle): PSUM drains are
    split by the free dim across the scalar and vector engines; the
    x fold-in runs on vector (bf16 2x), the sel mask on the scalar
    engine's activation-scale path.
  - z and the output are staged group-major ([NG, 128, G*1024]) by the
    host so each 4-block group is ONE contiguous DMA (8KB per-partition
    lines) - cuts the SP sequencer's DMA-issue cost 4x.
  - I/O is bf16, halving HBM traffic; x/y ride along as a small fp32
    sidecar. The host upcasts the bf16 result to fp32.
"""

import numpy as np
import ml_dtypes

import concourse.bacc as bacc
import concourse.bass as bass
import concourse.mybir as mybir
import concourse.tile as tile
from concourse.bass_utils import run_bass_kernel_spmd

B = 8
T = 4096
F = 1026
U = 1024
L = 128
G = 4  # blocks per scan batch / DMA group
NB = T // L  # 32 blocks
NG = NB // G  # 8 groups
CW = 64  # carry matmul contraction width (matmul bases must be 0/32/64)
f32 = mybir.dt.float32
bf16 = mybir.dt.bfloat16
nbf16 = ml_dtypes.bfloat16


def build_nc() -> bass.Bass:
    nc = bacc.Bacc()
    # group-major tiled: zin4[g, p, j*U + c] = z_{g*G*L + j*L + p, c}
    zin = nc.dram_tensor("zin", [NG, L, G * U], bf16, kind="ExternalInput")
    # yz[0, t] = y_t with block-start entries zeroed (scan reset)
    yz = nc.dram_tensor("yz", [1, T], bf16, kind="ExternalInput")
    # xct[p, k] = x_{k*L+p}: x staged with time in the partition dim
    xct = nc.dram_tensor("xct", [L, NB], f32, kind="ExternalInput")
    # e127y[p, k] = I[p==127] * y_{k*L}: sel mask+scale per block
    e127y = nc.dram_tensor("e127y", [L, NB], f32, kind="ExternalInput")
    out = nc.dram_tensor("out", [NG, L, G * U], bf16, kind="ExternalOutput")

    ident4_d = nc.inline_tensor(
        np.tile(np.eye(L, dtype=np.float32), (1, G)).astype(nbf16), name="ident4"
    )

    with tile.TileContext(nc) as tc:
        with (
            tc.tile_pool(name="const", bufs=1) as constp,
            tc.tile_pool(name="zpool", bufs=3) as zpool,
            tc.tile_pool(name="mtraw", bufs=2) as mtrawp,
            tc.tile_pool(name="mtpool", bufs=2) as mtp,
            tc.tile_pool(name="ybcpool", bufs=2) as ybcp,
            tc.tile_pool(name="pbcpool", bufs=2) as pbcp,
            tc.tile_pool(name="selpool", bufs=2) as selp,
            tc.tile_pool(name="otpool", bufs=3) as otp,
            tc.tile_pool(name="ps_out", bufs=3, space="PSUM") as psp,
        ):
            ident4 = constp.tile([L, G * L], bf16, tag="ident4")
            nc.sync.dma_start(ident4[:], ident4_d[:, :])
            xcol = constp.tile([L, NB], f32, tag="xcol")
            nc.sync.dma_start(xcol[:], xct[:, :])
            ecol = constp.tile([L, NB], f32, tag="ecol")
            nc.sync.dma_start(ecol[:], e127y[:, :])
            yzfull = constp.tile([1, T], bf16, tag="yz")
            nc.sync.dma_start(yzfull[:], yz[0:1, :])

            prev = None
            for g in range(NG):
                c0 = g * G * L
                tz4 = zpool.tile([L, G * U], bf16, tag="tz4")
                nc.sync.dma_start(tz4[:], zin[g, :, :])

                ybc4 = ybcp.tile([L, G * L], bf16, tag="ybc4")
                nc.gpsimd.partition_broadcast(ybc4[:], yzfull[0:1, c0 : c0 + G * L])

                # mt4raw[s, L*j + t] = prod_{r=s+1..t} y_r  (block g*G+j)
                mt4raw = mtrawp.tile([L, G * L], bf16, tag="mt4raw")
                nc.vector.tensor_tensor_scan(
                    mt4raw[:],
                    ybc4[:],
                    ident4[:],
                    0.0,
                    mybir.AluOpType.mult,
                    mybir.AluOpType.add,
                )
                # pbc4[p, L*j + t] = mt4raw[0, L*j + t]  (broadcast row 0)
                pbc4 = pbcp.tile([L, G * L], bf16, tag="pbc4")
                nc.gpsimd.partition_broadcast(pbc4[:], mt4raw[0:1, :])

                # fold x in on vector (bf16 2x): mt4 = x_s * mt4raw
                # sel4[s, L*j+t] = I[s==127] * y0_k * mt4raw[0, L*j+t]
                # (= I[s==127] * p_t) on scalar's activation-scale path
                mt4 = mtp.tile([L, G * L], bf16, tag="mt4")
                sel4 = selp.tile([L, G * L], bf16, tag="sel4")
                for j in range(G):
                    k = g * G + j
                    nc.vector.tensor_scalar_mul(
                        mt4[:, L * j : L * (j + 1)],
                        mt4raw[:, L * j : L * (j + 1)],
                        xcol[:, k : k + 1],
                    )
                    if k > 0:
                        nc.scalar.activation(
                            sel4[:, L * j : L * (j + 1)],
                            pbc4[:, L * j : L * (j + 1)],
                            mybir.ActivationFunctionType.Copy,
                            scale=ecol[:, k : k + 1],
                        )

                ot4 = otp.tile([L, G * U], bf16, tag="ot4")
                for j in range(G):
                    k = g * G + j
                    zb = j * U
                    po = psp.tile([L, U], f32, tag="po")
                    for jj in (0, 512):
                        nc.tensor.matmul(
                            po[:, jj : jj + 512],
                            mt4[:, L * j : L * (j + 1)],
                            tz4[:, zb + jj : zb + jj + 512],
                            start=True,
                            stop=(k == 0),
                        )
                    if k > 0:
                        # po[t, :] += p_t * prev[127, :]
                        for jj in (0, 512):
                            nc.tensor.matmul(
                                po[:, jj : jj + 512],
                                sel4[L - CW : L, L * j : L * (j + 1)],
                                prev[L - CW : L, jj : jj + 512],
                                start=False,
                                stop=True,
                            )
                    # drain split by the free dim: one half per engine
                    nc.scalar.copy(ot4[:, zb : zb + 512], po[:, 0:512])
                    nc.vector.tensor_copy(ot4[:, zb + 512 : zb + U], po[:, 512:1024])
                    prev = ot4[:, zb : zb + U]
                nc.sync.dma_start(out[g, :, :], ot4[:])
    nc.finalize()
    return nc


_NC = None


def _get_nc() -> bass.Bass:
    global _NC
    if _NC is None:
        _NC = build_nc()
    return _NC


def prep_in_maps(x: np.ndarray) -> list[dict]:
    xs = x[:, :, 0]  # [B,T]
    ys = x[:, :, 1]  # [B,T]
    # group-major tiling: [B, NG, L, G*U]
    zb = (
        np.ascontiguousarray(x[:, :, 2:])
        .astype(nbf16)
        .reshape(B, NG, G, L, U)
        .transpose(0, 1, 3, 2, 4)
        .reshape(B, NG, L, G * U)
    )
    zb = np.ascontiguousarray(zb)

    mask0 = (np.arange(T) % L) == 0
    yz = np.where(mask0[None, :], 0.0, ys).astype(nbf16)[:, None, :]
    xct = np.ascontiguousarray(xs.reshape(B, NB, L).transpose(0, 2, 1))
    e127y = np.zeros((B, L, NB), dtype=np.float32)
    e127y[:, L - 1, :] = ys[:, ::L]

    return [
        {"zin": zb[c], "yz": yz[c], "xct": xct[c], "e127y": e127y[c]}
        for c in range(B)
    ]


def unpack_out(outb: np.ndarray) -> np.ndarray:
    # [B, NG, L, G*U] -> [B, T, U]
    return (
        outb.reshape(B, NG, L, G, U)
        .transpose(0, 1, 3, 2, 4)
        .reshape(B, T, U)
        .astype(np.float32)
    )


def kernel(**inputs: np.ndarray) -> np.ndarray:
    x = np.ascontiguousarray(inputs["inputs"], dtype=np.float32)
    assert x.shape == (B, T, F), x.shape
    nc = _get_nc()
    res = run_bass_kernel_spmd(nc, prep_in_maps(x), core_ids=list(range(B)))
    outb = np.stack([res.results[c]["out"] for c in range(B)], axis=0)
    return unpack_out(outb)


# revision 13
# speedup vs baseline: 1.0808x; 1.0808x over previous
"""Trainium2 Bass kernel for CLSProcess: diagonal linear recurrence
state_t = y_t * state_{t-1} + x_t * z_t over [B=8, T=4096, units=1024].

Sharding: batch across the 8 cores (one batch element per core); the
recurrence is handled per-core with a chunked scan:
  - time is cut into 32 blocks of L=128 steps (partition dim = time)
  - per block, the decay matrix M'[t,s] = x_s * prod_{r=s+1..t} y_r
    (0 for s>t) is built EXACTLY with a DVE tensor_tensor_scan over the
    identity: state_s(t) = y_t*state + I[s==t]  =>  out[s,t] = M[t,s]
    (the lhsT layout the PE matmul wants), then scaled per-partition by
    x_s (x is staged time-major in the partition dim by the host).
    Scans are batched 4 blocks per instruction ([128,512]) with the y
    at block boundaries zeroed so the running state resets at each
    block start.
  - block output = M' @ z  (PE matmul, bf16 operands, fp32 PSUM) +
    carry term
  - carry term: po[t,:] += p_t * prev[127,:] with
    p_t = prod_{r=block_start..t} y_r = y_block_start * mt4raw[0, t]
    (no separate scan needed). Engines can only address partition
    bases {0,32,64,96} and matmul bases {0,32,64}, so row 127 of the
    previous block is reached with a K=64 matmul: sel[s,t] =
    I[s==127] * p_t (rows [64:128] used) against prev[64:128,:].
  - scheduling: block k+1's main matmuls are emitted BEFORE block k's
    carry matmuls so the in-order PE queue is never head-of-line
    blocked by the serial carry chain (PSUM pool is 4 deep = all 8
    banks). PSUM drains are split by the free dim across the scalar
    and vector engines.
  - z and the output are staged 2-block-major ([16, 128, 2*1024]) by
    the host so DMAs move 512KB with 4KB per-partition lines - halves
    the SP sequencer's DMA-issue cost.
  - I/O is bf16, halving HBM traffic; x/y ride along as a small fp32
    sidecar so the decay products stay full precision. The host
    upcasts the bf16 result to fp32.
"""

import numpy as np
import ml_dtypes

import concourse.bacc as bacc
import concourse.bass as bass
import concourse.mybir as mybir
import concourse.tile as tile
from concourse.bass_utils import run_bass_kernel_spmd

B = 8
T = 4096
F = 1026
U = 1024
L = 128
G = 4  # blocks per scan batch
D = 2  # blocks per DMA batch
NB = T // L  # 32 blocks
NG = NB // G  # 8 scan groups
ND = NB // D  # 16 DMA groups
CW = 64  # carry matmul contraction width (matmul bases must be 0/32/64)
SPLIT = 384  # drain free-dim split point: scalar [0:SPLIT], vector [SPLIT:]
f32 = mybir.dt.float32
bf16 = mybir.dt.bfloat16
nbf16 = ml_dtypes.bfloat16


def build_nc() -> bass.Bass:
    nc = bacc.Bacc()
    # 2-block-major tiled: zin[d, p, j*U + c] = z_{(d*D + j)*L + p, c}
    zin = nc.dram_tensor("zin", [ND, L, D * U], bf16, kind="ExternalInput")
    # yz[0, t] = y_t with block-start entries zeroed (scan reset)
    yz = nc.dram_tensor("yz", [1, T], f32, kind="ExternalInput")
    # xct[p, k] = x_{k*L+p}: x staged with time in the partition dim
    xct = nc.dram_tensor("xct", [L, NB], f32, kind="ExternalInput")
    # e127y[p, k] = I[p==127] * y_{k*L}: sel mask+scale per block
    e127y = nc.dram_tensor("e127y", [L, NB], f32, kind="ExternalInput")
    out = nc.dram_tensor("out", [ND, L, D * U], bf16, kind="ExternalOutput")

    ident4_d = nc.inline_tensor(
        np.tile(np.eye(L, dtype=np.float32), (1, G)), name="ident4"
    )

    with tile.TileContext(nc) as tc:
        with (
            tc.tile_pool(name="const", bufs=1) as constp,
            tc.tile_pool(name="zpool", bufs=5) as zpool,
            tc.tile_pool(name="mtraw", bufs=2) as mtrawp,
            tc.tile_pool(name="mtpool", bufs=2) as mtp,
            tc.tile_pool(name="ybcpool", bufs=2) as ybcp,
            tc.tile_pool(name="pbcpool", bufs=2) as pbcp,
            tc.tile_pool(name="selpool", bufs=2) as selp,
            tc.tile_pool(name="otpool", bufs=4) as otp,
            tc.tile_pool(name="ps_out", bufs=4, space="PSUM") as psp,
        ):
            ident4 = constp.tile([L, G * L], f32, tag="ident4")
            nc.sync.dma_start(ident4[:], ident4_d[:, :])
            xcol = constp.tile([L, NB], f32, tag="xcol")
            nc.sync.dma_start(xcol[:], xct[:, :])
            ecol = constp.tile([L, NB], f32, tag="ecol")
            nc.sync.dma_start(ecol[:], e127y[:, :])
            yzfull = constp.tile([1, T], f32, tag="yz")
            nc.sync.dma_start(yzfull[:], yz[0:1, :])

            # group-level prep (scan, x fold-in, sel build) for group g
            mt4s, sel4s = {}, {}

            def prep_group(g):
                c0 = g * G * L
                ybc4 = ybcp.tile([L, G * L], f32, tag="ybc4")
                nc.gpsimd.partition_broadcast(
                    ybc4[:], yzfull[0:1, c0 : c0 + G * L]
                )
                # mt4raw[s, L*j + t] = prod_{r=s+1..t} y_r  (block g*G+j)
                mt4raw = mtrawp.tile([L, G * L], f32, tag="mt4raw")
                nc.vector.tensor_tensor_scan(
                    mt4raw[:],
                    ybc4[:],
                    ident4[:],
                    0.0,
                    mybir.AluOpType.mult,
                    mybir.AluOpType.add,
                )
                # pbc4[p, L*j + t] = mt4raw[0, L*j + t]  (broadcast row 0)
                pbc4 = pbcp.tile([L, G * L], f32, tag="pbc4")
                nc.gpsimd.partition_broadcast(pbc4[:], mt4raw[0:1, :])

                mt4 = mtp.tile([L, G * L], bf16, tag="mt4")
                sel4 = selp.tile([L, G * L], bf16, tag="sel4")
                for j in range(G):
                    k = g * G + j
                    nc.scalar.activation(
                        mt4[:, L * j : L * (j + 1)],
                        mt4raw[:, L * j : L * (j + 1)],
                        mybir.ActivationFunctionType.Copy,
                        scale=xcol[:, k : k + 1],
                    )
                    if k > 0:
                        nc.scalar.activation(
                            sel4[:, L * j : L * (j + 1)],
                            pbc4[:, L * j : L * (j + 1)],
                            mybir.ActivationFunctionType.Copy,
                            scale=ecol[:, k : k + 1],
                        )
                mt4s[g], sel4s[g] = mt4, sel4

            # per-block state for the software-pipelined emission
            tzs, pos, ots = {}, {}, {}

            def emit_load(k):
                if k % D == 0:
                    d = k // D
                    tz = zpool.tile([L, D * U], bf16, tag="tz")
                    nc.sync.dma_start(tz[:], zin[d, :, :])
                    for jj in range(D):
                        tzs[k + jj] = tz[:, jj * U : (jj + 1) * U]

            def emit_main(k):
                g, j = k // G, k % G
                if j == 0:
                    prep_group(g)
                po = psp.tile([L, U], f32, tag="po")
                for jj in (0, 512):
                    nc.tensor.matmul(
                        po[:, jj : jj + 512],
                        mt4s[g][:, L * j : L * (j + 1)],
                        tzs[k][:, jj : jj + 512],
                        start=True,
                        stop=(k == 0),
                    )
                pos[k] = po

            def emit_carry_and_drain(k):
                g, j = k // G, k % G
                po = pos.pop(k)
                if k > 0:
                    prev = ots[k - 1]
                    # po[t, :] += p_t * prev[127, :]
                    for jj in (0, 512):
                        nc.tensor.matmul(
                            po[:, jj : jj + 512],
                            sel4s[g][L - CW : L, L * j : L * (j + 1)],
                            prev[L - CW : L, jj : jj + 512],
                            start=False,
                            stop=True,
                        )
                if k % D == 0:
                    d = k // D
                    ot = otp.tile([L, D * U], bf16, tag="ot")
                    for jj in range(D):
                        ots[k + jj] = ot[:, jj * U : (jj + 1) * U]
                    ots[(d, "tile")] = ot
                otk = ots[k]
                # drain split by the free dim: one piece per engine
                nc.scalar.copy(otk[:, 0:SPLIT], po[:, 0:SPLIT])
                nc.vector.tensor_copy(otk[:, SPLIT:U], po[:, SPLIT:U])
                if k % D == D - 1:
                    d = k // D
                    nc.sync.dma_start(out[d, :, :], ots.pop((d, "tile"))[:])

            # software pipeline: mains run one block ahead of carries so
            # the PE queue always has independent work while the carry
            # chain waits on the previous block's drain
            emit_load(0)
            emit_main(0)
            for k in range(1, NB):
                emit_load(k)
                emit_main(k)
                emit_carry_and_drain(k - 1)
            emit_carry_and_drain(NB - 1)
    nc.finalize()
    return nc


_NC = None


def _get_nc() -> bass.Bass:
    global _NC
    if _NC is None:
        _NC = build_nc()
    return _NC


def prep_in_maps(x: np.ndarray) -> list[dict]:
    xs = x[:, :, 0]  # [B,T]
    ys = x[:, :, 1]  # [B,T]
    # 2-block-major tiling: [B, ND, L, D*U]
    zb = (
        np.ascontiguousarray(x[:, :, 2:])
        .astype(nbf16)
        .reshape(B, ND, D, L, U)
        .transpose(0, 1, 3, 2, 4)
        .reshape(B, ND, L, D * U)
    )
    zb = np.ascontiguousarray(zb)

    mask0 = (np.arange(T) % L) == 0
    yz = np.where(mask0[None, :], 0.0, ys).astype(np.float32)[:, None, :]
    xct = np.ascontiguousarray(xs.reshape(B, NB, L).transpose(0, 2, 1))
    e127y = np.zeros((B, L, NB), dtype=np.float32)
    e127y[:, L - 1, :] = ys[:, ::L]

    return [
        {"zin": zb[c], "yz": yz[c], "xct": xct[c], "e127y": e127y[c]}
        for c in range(B)
    ]


def unpack_out(outb: np.ndarray) -> np.ndarray:
    # [B, ND, L, D*U] -> [B, T, U]
    return (
        outb.reshape(B, ND, L, D, U)
        .transpose(0, 1, 3, 2, 4)
        .reshape(B, T, U)
        .astype(np.float32)
    )


def kernel(**inputs: np.ndarray) -> np.ndarray:
    x = np.ascontiguousarray(inputs["inputs"], dtype=np.float32)
    assert x.shape == (B, T, F), x.shape
    nc = _get_nc()
    res = run_bass_kernel_spmd(nc, prep_in_maps(x), core_ids=list(range(B)))
    outb = np.stack([res.results[c]["out"] for c in range(B)], axis=0)
    return unpack_out(outb)
